# revision 16
# baseline (speedup 1.0000x reference)
"""NerfExperts MoE kernel for Trainium2, expert-parallel over 8 NeuronCores.

Strategy: each of the 1024 points is routed to one of 100 experts
(~2.3MB of fp32 weights each -> memory bound).  Experts are sharded
across the 8 cores (13 slots per core), tokens dispatched on the host,
and each expert's weights stream from HBM exactly once.  Weight regions
that tolerate it are stored as fp8 e3m4 scaled by 64 (descale is fused
into the PSUM->SBUF bias-add); the precision-sensitive late layers
(w5/w6/w7 by default) stay bf16.  Weights are shipped LAYER-MAJOR in
per-(region, wave) pieces, each split across the two HWDGE rings, in
exact consumption order - compute for stage l wave w only waits on its
own ~0.25-0.9MB piece, so the DMA stream and the stage pipeline overlap
tightly from ~1us onward.  Activations stay transposed ([feature,
token]); experts advance through the MLP in lockstep waves sharing PSUM
tiles, with per-expert fp32 biases applied via stride-0 broadcast APs on
DVE and relu on ACT.  Harmonic-embedding phases are computed in fp32
with Cody-Waite range reduction for Sin.

Embedding tile rows: points: sin 0:18, cos 32:50, xyz 50:53 (53 rows);
dirs: sin 0:12, cos 32:44, xyz 44:47 (47 rows).  Dead rows are zero in
the weight slabs, so garbage there is harmless.
"""

import numpy as np
import ml_dtypes

import concourse.bass as bass
import concourse.bacc as bacc
import concourse.mybir as mybir
import concourse.tile as tile
from concourse.bass_utils import run_bass_kernel_spmd

PI = float(np.pi)
N_CORES = 8
E = 100
NX, ND = 6, 4
CAP_MAX = 128  # max tokens per expert slot (keeps matmul N and PSUM in range)
NB = 21       # bias table columns per slot (9 stages x2, ba, bc0, bc1)

FP8_SCALE = 64.0
# regions stored as fp8 e3m4 (x64); the rest are bf16
FP8R = frozenset(["l0", "w1", "w2", "w3", "w4", "w5", "w7", "wi", "wc0", "rays"])
# bf16 regions pre-scaled x64 because they share a PSUM accumulation with an
# fp8 region (stage 5: w5 main fp8 + skip bf16)
SCALED_BF16 = frozenset(["skip"]) if "w5" in FP8R else frozenset()

# region -> (rows, cols per slot); "hd" merges wa (cols 0:2) + wc1 (2:5)
REGIONS = {
    "l0":   (53, 256),
    "w1":   (128, 512), "w2": (128, 512), "w3": (128, 512), "w4": (128, 512),
    "w5":   (128, 512), "skip": (53, 256),
    "w6":   (128, 512), "w7":   (128, 512),
    "wi":   (128, 512), "hd":   (128, 5),
    "wc0":  (128, 256), "rays": (47, 128),
}
# weight-DMA / consumption order (one DMA per region: descriptor generation
# on the sequencer costs ~0.6us per 128-row DMA, so fewer+larger wins)
REGION_ORDER = ["l0", "w1", "w2", "w3", "w4", "w5", "skip", "w6", "w7",
                "wi", "hd", "wc0", "rays"]


def _make_waves(nslot, C):
    gmax = max(1, min(512 // (2 * C), 8))
    nw = int(np.ceil(nslot / gmax))
    base = nslot // nw
    rem = nslot - base * nw
    sizes = [base + (1 if i < rem else 0) for i in range(nw)]
    waves, s0 = [], 0
    for g in sizes:
        waves.append((s0, s0 + g))
        s0 += g
    return waves


def _pack_expert(reg, bt, s, nslot, inputs, e, waves):
    """Fill slot s of the per-region fp32 arrays and bias table.

    Paired biases are stored j-major per wave: for stage lidx and wave
    (s0, s1), cols [lidx*2*nslot + 2*s0 : ... + 2*s1] hold
    [b_lo(s0..s1), b_hi(s0..s1)] so a wave's bias is one contiguous
    [128, 2g] block (3D-broadcastable along C).
    """
    n2 = 2 * nslot
    s0, s1 = next(w for w in waves if w[0] <= s < w[1])
    g = s1 - s0

    def set_b2(lidx, b):
        base = lidx * n2 + 2 * s0 + (s - s0)
        bt[:, base] = b[0:128]
        bt[:, base + g] = b[128:256]

    def emb_rows(dst, w, o, ncol, nsin):
        dst[0:nsin, o: o + ncol] = w[0:nsin]
        dst[32:32 + nsin, o: o + ncol] = w[nsin:2 * nsin]
        dst[32 + nsin: 32 + nsin + 3, o: o + ncol] = w[2 * nsin: 2 * nsin + 3]

    emb_rows(reg["l0"], inputs["w0"][e], s * 256, 256, 18)
    set_b2(0, inputs["b0"][e])
    w5 = inputs["w5"][e]
    emb_rows(reg["skip"], w5[256:295], s * 256, 256, 18)
    emb_rows(reg["rays"], inputs["wc0"][e][256:283], s * 128, 128, 12)
    for l in (1, 2, 3, 4, 6, 7):
        w = inputs[f"w{l}"][e]
        o = s * 512
        for k in (0, 1):
            reg[f"w{l}"][:, o + k * 256: o + (k + 1) * 256] = w[128 * k: 128 * (k + 1)]
        set_b2(l, inputs[f"b{l}"][e])
    o = s * 512
    for k in (0, 1):
        reg["w5"][:, o + k * 256: o + (k + 1) * 256] = w5[128 * k: 128 * (k + 1)]
    set_b2(5, inputs["b5"][e])
    wi = inputs["wi"][e]
    for k in (0, 1):
        reg["wi"][:, o + k * 256: o + (k + 1) * 256] = wi[128 * k: 128 * (k + 1)]
    set_b2(8, inputs["bi"][e])
    wa = inputs["wa"][e][:, 0]
    reg["hd"][:, s * 5] = wa[0:128]
    reg["hd"][:, s * 5 + 1] = wa[128:256]
    bt[0, 18 * nslot + s] = inputs["ba"][e][0]
    wc0 = inputs["wc0"][e]
    reg["wc0"][:, s * 256: s * 256 + 128] = wc0[0:128]
    reg["wc0"][:, s * 256 + 128: s * 256 + 256] = wc0[128:256]
    bt[:, 19 * nslot + s] = inputs["bc0"][e]
    reg["hd"][:, s * 5 + 2: s * 5 + 5] = inputs["wc1"][e]
    bt[0:3, 20 * nslot + s] = inputs["bc1"][e]


# ---------------------------------------------------------------------------
# Device program
# ---------------------------------------------------------------------------

def _build_program(C, nslot):
    nall = nslot * C
    waves = _make_waves(nslot, C)
    nw = len(waves)
    f32 = mybir.dt.float32
    bf16 = mybir.dt.bfloat16
    f8e3 = mybir.dt.float8e3
    Sin = mybir.ActivationFunctionType.Sin
    Sigmoid = mybir.ActivationFunctionType.Sigmoid
    Relu = mybir.ActivationFunctionType.Relu
    ADD = mybir.AluOpType.add
    SUB = mybir.AluOpType.subtract
    MUL = mybir.AluOpType.mult
    MAX = mybir.AluOpType.max
    MIN = mybir.AluOpType.min
    INV2PI = float(np.float32(1.0 / (2 * PI)))
    MAGIC = 12582912.0            # 1.5 * 2**23: forces round-to-int in fp32
    C1 = 6.28125                  # 2*pi high part, exact in fp32
    C2 = float(np.float32(2 * PI - 6.28125))
    CLAMP = 3.1415925             # just under pi (ACT Sin domain is [-pi, pi])
    HALF_PI = float(np.float32(PI / 2))
    DESC = float(1.0 / FP8_SCALE)

    def rdt(r):
        return f8e3 if r in FP8R else bf16

    nc = bacc.Bacc("TRN2", target_bir_lowering=False, debug=False)
    wt_d = {}
    for r in REGION_ORDER:
        rows, cols = REGIONS[r]
        wt_d[r] = nc.dram_tensor(f"wt_{r}", (rows, nslot * cols), rdt(r),
                                 kind="ExternalInput")
    bt_d = nc.dram_tensor("bt", (128, NB * nslot), f32, kind="ExternalInput")
    # sm3 rows: [fx 0:18 | fd 18:30 | ptsT 30:30+nall | dirT ...+nall]
    sm3_d = nc.dram_tensor("sm3", (3, 30 + 2 * nall), f32, kind="ExternalInput")
    xyz_d = nc.dram_tensor("xyzb", (6, nall), bf16, kind="ExternalInput")
    al_d = nc.dram_tensor("alpha_out", (1, nall), f32, kind="ExternalOutput")
    co_d = nc.dram_tensor("color_out", (3, nall), f32, kind="ExternalOutput")

    with tile.TileContext(nc) as tc:
        with (
            tc.tile_pool(name="cp", bufs=1) as cp,
            tc.tile_pool(name="xp", bufs=2 * nw + 2) as xp,
            tc.tile_pool(name="psA", bufs=6, space=bass.MemorySpace.PSUM) as psA,
            tc.tile_pool(name="psB", bufs=2, space=bass.MemorySpace.PSUM) as psB,
        ):
            embP = cp.tile([53, nall], bf16)   # sin 0:18, cos 32:50, xyz 50:53
            embD = cp.tile([47, nall], bf16)   # sin 0:12, cos 32:44, xyz 44:47
            nc.vector.memset(embP[:], 0.0)
            nc.vector.memset(embD[:], 0.0)
            # small inputs: scalar ring first, then weight pieces on both rings
            sm3 = cp.tile([3, 30 + 2 * nall], f32)
            nc.scalar.dma_start(sm3[:], sm3_d.ap()[:])
            nc.scalar.dma_start(embP[50:53, :], xyz_d.ap()[0:3, :])
            nc.scalar.dma_start(embD[44:47, :], xyz_d.ap()[3:6, :])
            bt_sb = cp.tile([128, NB * nslot], f32)
            nc.scalar.dma_start(bt_sb[:], bt_d.ap()[:])
            fx_sb = sm3[:, 0:18]
            fd_sb = sm3[:, 18:30]
            pts_sb = sm3[:, 30:30 + nall]
            dir_sb = sm3[:, 30 + nall:30 + 2 * nall]

            # ---- weight pieces, consumption order, both rings balanced ----
            wts = {}
            for r in REGION_ORDER:
                rows, cols = REGIONS[r]
                wts[r] = cp.tile([rows, nslot * cols], rdt(r),
                                 name=f"wt_{r}", tag=f"wt_{r}")
            # all weight DMAs on the sync HWDGE ring, in consumption order:
            # the scalar engine must stay free for ACT work (Sin/Relu), and a
            # single InstDMACopy already spreads across all 16 SDMA engines.
            for r in REGION_ORDER:
                nc.sync.dma_start(wts[r][:], wt_d[r].ap()[:])

            def slab(r, s, lo, hi, wv, rows=128):
                cols = REGIONS[r][1]
                o = s * cols
                return wts[r][0:rows, o + lo: o + hi]

            alpha_sb = cp.tile([1, nall], f32)
            color_sb = cp.tile([3, nall], f32)

            # frequency expansion + range-reduced sin/cos, in <=512-col chunks
            def reduce_sin(tsrc, rows, ncol):
                t1 = xp.tile([rows, ncol], f32, tag="vred")
                nc.vector.tensor_scalar(t1[:], tsrc, INV2PI, MAGIC, MUL, ADD)
                r = xp.tile([rows, ncol], f32, tag="vred")
                nc.vector.tensor_scalar(r[:], t1[:], MAGIC, None, SUB)
                rd = xp.tile([rows, ncol], f32, tag="vred")
                nc.vector.scalar_tensor_tensor(rd[:], r[:], -C1, tsrc, MUL, ADD)
                rd2 = xp.tile([rows, ncol], f32, tag="vred")
                nc.vector.scalar_tensor_tensor(rd2[:], r[:], -C2, rd[:], MUL, ADD)
                v = xp.tile([rows, ncol], f32, tag="vred")
                nc.vector.tensor_scalar(v[:], rd2[:], CLAMP, -CLAMP, MIN, MAX)
                return v

            for lo in range(0, nall, 512):
                hi = min(nall, lo + 512)
                w_ = hi - lo
                for (rows, fmat, src, dst) in (
                    (18, fx_sb, pts_sb, embP),
                    (12, fd_sb, dir_sb, embD),
                ):
                    ep = psA.tile([rows, w_], f32, tag="mlp")
                    nc.tensor.matmul(ep[:], fmat[:, 0:rows], src[:, lo:hi],
                                     start=True, stop=True)
                    vs = reduce_sin(ep[:], rows, w_)
                    nc.scalar.activation(dst[0:rows, lo:hi], vs[:], Sin)
                    pre = xp.tile([rows, w_], f32, tag="vred")
                    nc.vector.tensor_scalar(pre[:], ep[:], HALF_PI, None, ADD)
                    vc = reduce_sin(pre[:], rows, w_)
                    cs = xp.tile([rows, w_], f32, tag="vred")
                    nc.scalar.activation(cs[:], vc[:], Sin)
                    nc.vector.tensor_copy(dst[32:32 + rows, lo:hi], cs[:])

            # ---- wave-lockstep MLP ----
            def bias2_bcast(lidx, s0, s1):
                g = s1 - s0
                ap = bt_sb[:, lidx * 2 * nslot + s0 * 2: lidx * 2 * nslot + s1 * 2]
                return ap.broadcast_to([128, 2 * g, C])

            def bias1_bcast(which, s0, s1, p=128):
                g = s1 - s0
                ap = bt_sb[0:p, which * nslot + s0: which * nslot + s1]
                return ap.broadcast_to([p, g, C])

            xs = [None] * nw
            its = [None] * nw
            cts = [None] * nw

            def mm_mid(r, ps, xin, s0, s1, wv):
                for i in range(s1 - s0):
                    s = s0 + i
                    for j in (0, 1):
                        pj = ps[:, j, i * C:(i + 1) * C]
                        nc.tensor.matmul(pj, slab(r, s, j * 128, j * 128 + 128, wv),
                                         xin[:, 0, i * C:(i + 1) * C],
                                         start=True, stop=False)
                        nc.tensor.matmul(pj, slab(r, s, 256 + j * 128, 256 + j * 128 + 128, wv),
                                         xin[:, 1, i * C:(i + 1) * C],
                                         start=False, stop=True)

            def move2(ps, lidx, s0, s1, wv, relu=True, fp8=False):
                g = s1 - s0
                xn = xp.tile([128, 2, g * C], bf16, tag="x")
                psv = ps[:].rearrange("p j (g c) -> p (j g) c", g=g)
                xnv = xn[:].rearrange("p j (g c) -> p (j g) c", g=g)
                if fp8:
                    nc.vector.scalar_tensor_tensor(
                        xnv, psv, DESC, bias2_bcast(lidx, s0, s1), MUL, ADD)
                else:
                    nc.vector.tensor_tensor(xnv, psv, bias2_bcast(lidx, s0, s1), ADD)
                if relu:
                    nc.scalar.activation(xn[:], xn[:], Relu)
                return xn

            def emit_stage(wv, stage):
                s0, s1 = waves[wv]
                g = s1 - s0
                if stage == 0:  # L0
                    ps = psA.tile([128, 2, g * C], f32, tag="mlp")
                    for i in range(g):
                        s = s0 + i
                        sl = slice(s * C, (s + 1) * C)
                        for j in (0, 1):
                            nc.tensor.matmul(ps[:, j, i * C:(i + 1) * C],
                                             slab("l0", s, j * 128, j * 128 + 128, wv, rows=53),
                                             embP[0:53, sl],
                                             start=True, stop=True)
                    xs[wv] = move2(ps, 0, s0, s1, wv, fp8="l0" in FP8R)
                elif stage in (1, 2, 3, 4, 6, 7):
                    r = f"w{stage}"
                    ps = psA.tile([128, 2, g * C], f32, tag="mlp")
                    mm_mid(r, ps, xs[wv], s0, s1, wv)
                    xs[wv] = move2(ps, stage, s0, s1, wv, fp8=r in FP8R)
                elif stage == 5:
                    ps = psA.tile([128, 2, g * C], f32, tag="mlp")
                    xin = xs[wv]
                    for i in range(g):
                        s = s0 + i
                        sl = slice(s * C, (s + 1) * C)
                        for j in (0, 1):
                            pj = ps[:, j, i * C:(i + 1) * C]
                            nc.tensor.matmul(pj, slab("w5", s, j * 128, j * 128 + 128, wv),
                                             xin[:, 0, i * C:(i + 1) * C],
                                             start=True, stop=False)
                            nc.tensor.matmul(pj, slab("w5", s, 256 + j * 128, 256 + j * 128 + 128, wv),
                                             xin[:, 1, i * C:(i + 1) * C],
                                             start=False, stop=False)
                            nc.tensor.matmul(pj, slab("skip", s, j * 128, j * 128 + 128, wv, rows=53),
                                             embP[0:53, sl],
                                             start=False, stop=True)
                    xs[wv] = move2(ps, 5, s0, s1, wv, fp8="w5" in FP8R)
                elif stage == 8:  # wi -> inter (bias, no relu)
                    ps = psA.tile([128, 2, g * C], f32, tag="mlp")
                    mm_mid("wi", ps, xs[wv], s0, s1, wv)
                    its[wv] = move2(ps, 8, s0, s1, wv, relu=False, fp8="wi" in FP8R)
                elif stage == 9:  # wa -> alpha
                    pa = psB.tile([3, g * C], f32, tag="head")
                    xin = xs[wv]
                    for i in range(g):
                        s = s0 + i
                        nc.tensor.matmul(pa[0:1, i * C:(i + 1) * C],
                                         slab("hd", s, 0, 1, wv),
                                         xin[:, 0, i * C:(i + 1) * C],
                                         start=True, stop=False)
                        nc.tensor.matmul(pa[0:1, i * C:(i + 1) * C],
                                         slab("hd", s, 1, 2, wv),
                                         xin[:, 1, i * C:(i + 1) * C],
                                         start=False, stop=True)
                    av = alpha_sb[0:1, s0 * C: s1 * C].rearrange(
                        "p (g c) -> p g c", g=g)
                    pav = pa[0:1, :].rearrange("p (g c) -> p g c", g=g)
                    nc.vector.tensor_tensor(av, pav, bias1_bcast(18, s0, s1, p=1), ADD)
                elif stage == 10:  # wc0 + rays -> c (relu)
                    pc = psA.tile([128, g * C], f32, tag="mlp")
                    it = its[wv]
                    for i in range(g):
                        s = s0 + i
                        sl = slice(s * C, (s + 1) * C)
                        pj = pc[:, i * C:(i + 1) * C]
                        nc.tensor.matmul(pj, slab("wc0", s, 0, 128, wv),
                                         it[:, 0, i * C:(i + 1) * C],
                                         start=True, stop=False)
                        nc.tensor.matmul(pj, slab("wc0", s, 128, 256, wv),
                                         it[:, 1, i * C:(i + 1) * C],
                                         start=False, stop=False)
                        nc.tensor.matmul(pj, slab("rays", s, 0, 128, wv, rows=47),
                                         embD[0:47, sl],
                                         start=False, stop=True)
                    ct = xp.tile([128, g * C], bf16, tag="ct")
                    pcv = pc[:].rearrange("p (g c) -> p g c", g=g)
                    ctv = ct[:].rearrange("p (g c) -> p g c", g=g)
                    if "wc0" in FP8R:
                        nc.vector.scalar_tensor_tensor(
                            ctv, pcv, DESC, bias1_bcast(19, s0, s1), MUL, ADD)
                    else:
                        nc.vector.tensor_tensor(ctv, pcv, bias1_bcast(19, s0, s1), ADD)
                    nc.scalar.activation(ct[:], ct[:], Relu)
                    cts[wv] = ct
                elif stage == 11:  # wc1 -> sigmoid color
                    pcol = psB.tile([3, g * C], f32, tag="head")
                    ct = cts[wv]
                    for i in range(g):
                        s = s0 + i
                        nc.tensor.matmul(pcol[:, i * C:(i + 1) * C],
                                         slab("hd", s, 2, 5, wv),
                                         ct[:, i * C:(i + 1) * C],
                                         start=True, stop=True)
                    ctmp = xp.tile([3, g * C], f32, tag="ctmp")
                    pv = pcol[:].rearrange("p (g c) -> p g c", g=g)
                    cv = ctmp[:].rearrange("p (g c) -> p g c", g=g)
                    nc.vector.tensor_tensor(cv, pv, bias1_bcast(20, s0, s1, p=3), ADD)
                    nc.scalar.activation(color_sb[0:3, s0 * C: s1 * C], ctmp[:],
                                         Sigmoid)

            for stage in range(12):
                for wv in range(nw):
                    emit_stage(wv, stage)

            nc.sync.dma_start(al_d.ap()[:], alpha_sb[:])
            nc.scalar.dma_start(co_d.ap()[:], color_sb[:])

    nc.compile()
    return nc


_prog_cache = {}
_last_results = None


def _get_program(C, nslot):
    key = (C, nslot)
    if key not in _prog_cache:
        _prog_cache[key] = _build_program(C, nslot)
    return _prog_cache[key]


# ---------------------------------------------------------------------------
# Host wrapper
# ---------------------------------------------------------------------------

def kernel(**inputs):
    global _last_results
    inputs = {k: np.asarray(v) for k, v in inputs.items()}
    idx = inputs["index"].astype(np.int64)
    B = idx.shape[0]
    points = inputs["points"].astype(np.float32)
    dirs = inputs["directions"].astype(np.float32)

    # --- routing: split each expert's tokens into <=CAP_MAX chunks, round-
    # robin (sorted by size) over 8 cores ---
    tok = [np.nonzero(idx == e)[0] for e in range(E)]
    virt = []
    for e in range(E):
        t = tok[e]
        if len(t) == 0:
            continue
        for lo in range(0, len(t), CAP_MAX):
            virt.append((e, t[lo: lo + CAP_MAX]))
    if not virt:
        virt = [(0, np.zeros((0,), np.int64))]
    virt.sort(key=lambda v: -len(v[1]))
    nslot = max(1, int(np.ceil(len(virt) / N_CORES)))
    C = max(4, int(np.ceil(max(len(v[1]) for v in virt) / 4) * 4))
    nall = nslot * C

    core_slots = [[] for _ in range(N_CORES)]
    for i, v in enumerate(virt):
        core_slots[i % N_CORES].append(v)
    waves = _make_waves(nslot, C)

    nc = _get_program(C, nslot)

    fx = np.zeros((3, 18), np.float32)
    for c in range(3):
        for k in range(NX):
            fx[c, c * NX + k] = float(2 ** k)
    fd = np.zeros((3, 12), np.float32)
    for c in range(3):
        for k in range(ND):
            fd[c, c * ND + k] = float(2 ** k)

    in_maps = []
    for c in range(N_CORES):
        reg = {r: np.zeros((REGIONS[r][0], nslot * REGIONS[r][1]), np.float32)
               for r in REGION_ORDER}
        bt = np.zeros((128, NB * nslot), np.float32)
        ptsT = np.zeros((3, nall), np.float32)
        dirT = np.zeros((3, nall), np.float32)
        for s, (e, t) in enumerate(core_slots[c]):
            _pack_expert(reg, bt, s, nslot, inputs, e, waves)
            n = len(t)
            if n:
                ptsT[:, s * C: s * C + n] = points[t].T
                dirT[:, s * C: s * C + n] = dirs[t].T
        sm3 = np.concatenate([fx, fd, ptsT, dirT], axis=1)
        xyzb = np.concatenate([ptsT, dirT], axis=0).astype(ml_dtypes.bfloat16)
        im = {"bt": bt, "sm3": sm3, "xyzb": xyzb}
        for r in REGION_ORDER:
            if r in FP8R:
                im[f"wt_{r}"] = (reg[r] * FP8_SCALE).astype(ml_dtypes.float8_e3m4)
            elif r in SCALED_BF16:
                im[f"wt_{r}"] = (reg[r] * FP8_SCALE).astype(ml_dtypes.bfloat16)
            else:
                im[f"wt_{r}"] = reg[r].astype(ml_dtypes.bfloat16)
        in_maps.append(im)

    res = run_bass_kernel_spmd(nc, in_maps, core_ids=list(range(N_CORES)))
    _last_results = res

    out = np.zeros((B, 4), np.float32)
    for c in range(N_CORES):
        al = res.results[c]["alpha_out"]
        co = res.results[c]["color_out"]
        for s, (e, t) in enumerate(core_slots[c]):
            n = len(t)
            if n:
                out[t, 0] = al[0, s * C: s * C + n]
                out[t, 1:4] = co[:, s * C: s * C + n].T
    return out


# revision 17
# speedup vs baseline: 1.2691x; 1.2691x over previous
"""NerfExperts MoE kernel for Trainium2, expert-parallel over 8 NeuronCores.

Strategy: each of the 1024 points is routed to one of 100 experts
(~2.3MB of fp32 weights each -> memory bound).  Experts are sharded
across the 8 cores (13 slots per core), tokens dispatched on the host,
and each expert's weights stream from HBM exactly once.  Weight regions
that tolerate it are stored as fp8 e3m4 scaled by 64 (descale is fused
into the PSUM->SBUF bias-add); the precision-sensitive late layers
(w5/w6/w7 by default) stay bf16.  Weights are shipped LAYER-MAJOR in
per-(region, wave) pieces, each split across the two HWDGE rings, in
exact consumption order - compute for stage l wave w only waits on its
own ~0.25-0.9MB piece, so the DMA stream and the stage pipeline overlap
tightly from ~1us onward.  Activations stay transposed ([feature,
token]); experts advance through the MLP in lockstep waves sharing PSUM
tiles, with per-expert fp32 biases applied via stride-0 broadcast APs on
DVE and relu on ACT.  Harmonic-embedding phases are computed in fp32
with Cody-Waite range reduction for Sin.

Embedding tile rows: points: sin 0:18, cos 32:50, xyz 50:53 (53 rows);
dirs: sin 0:12, cos 32:44, xyz 44:47 (47 rows).  Dead rows are zero in
the weight slabs, so garbage there is harmless.
"""

import numpy as np
import ml_dtypes

import concourse.bass as bass
import concourse.bacc as bacc
import concourse.mybir as mybir
import concourse.tile as tile
from concourse.bass_utils import run_bass_kernel_spmd

PI = float(np.pi)
N_CORES = 8
E = 100
NX, ND = 6, 4
CAP_MAX = 128  # max tokens per expert slot (keeps matmul N and PSUM in range)
NB = 21       # bias table columns per slot (9 stages x2, ba, bc0, bc1)

FP8_SCALE = 64.0
# regions stored as fp8 e3m4 (x64); the rest are bf16
FP8R = frozenset(["l0", "w1", "w2", "w3", "w4", "w5", "w7", "wi", "wc0", "rays"])
# bf16 regions pre-scaled x64 because they share a PSUM accumulation with an
# fp8 region (stage 5: w5 main fp8 + skip bf16)
SCALED_BF16 = frozenset(["skip"]) if "w5" in FP8R else frozenset()

# region -> (rows, cols per slot); "hd" merges wa (cols 0:2) + wc1 (2:5).
# l0/skip/rays only use rows 0:53 / 0:47 but are padded to 128 rows: DMAs
# with <128 partition rows land on a single SDMA engine (~27GB/s) and
# serialize the whole FIFO ring behind them.
REGIONS = {
    "l0":   (128, 256),
    "w1":   (128, 512), "w2": (128, 512), "w3": (128, 512), "w4": (128, 512),
    "w5":   (128, 512), "skip": (128, 256),
    "w6":   (128, 512), "w7":   (128, 512),
    "wi":   (128, 512), "hd":   (128, 5),
    "wc0":  (128, 256), "rays": (128, 128),
}
# weight-DMA / consumption order (one DMA per region: descriptor generation
# on the sequencer costs ~0.6us per 128-row DMA, so fewer+larger wins)
REGION_ORDER = ["l0", "w1", "w2", "w3", "w4", "w5", "skip", "w6", "w7",
                "wi", "hd", "wc0", "rays"]


def _make_waves(nslot, C):
    gmax = max(1, min(512 // (2 * C), 8))
    nw = int(np.ceil(nslot / gmax))
    base = nslot // nw
    rem = nslot - base * nw
    sizes = [base + (1 if i < rem else 0) for i in range(nw)]
    waves, s0 = [], 0
    for g in sizes:
        waves.append((s0, s0 + g))
        s0 += g
    return waves


def _pack_expert(reg, bt, s, nslot, inputs, e, waves):
    """Fill slot s of the per-region fp32 arrays and bias table.

    Paired biases are stored j-major per wave: for stage lidx and wave
    (s0, s1), cols [lidx*2*nslot + 2*s0 : ... + 2*s1] hold
    [b_lo(s0..s1), b_hi(s0..s1)] so a wave's bias is one contiguous
    [128, 2g] block (3D-broadcastable along C).
    """
    n2 = 2 * nslot
    s0, s1 = next(w for w in waves if w[0] <= s < w[1])
    g = s1 - s0

    def set_b2(lidx, b):
        base = lidx * n2 + 2 * s0 + (s - s0)
        bt[:, base] = b[0:128]
        bt[:, base + g] = b[128:256]

    def emb_rows(dst, w, o, ncol, nsin):
        dst[0:nsin, o: o + ncol] = w[0:nsin]
        dst[32:32 + nsin, o: o + ncol] = w[nsin:2 * nsin]
        dst[32 + nsin: 32 + nsin + 3, o: o + ncol] = w[2 * nsin: 2 * nsin + 3]

    emb_rows(reg["l0"], inputs["w0"][e], s * 256, 256, 18)
    set_b2(0, inputs["b0"][e])
    w5 = inputs["w5"][e]
    emb_rows(reg["skip"], w5[256:295], s * 256, 256, 18)
    emb_rows(reg["rays"], inputs["wc0"][e][256:283], s * 128, 128, 12)
    for l in (1, 2, 3, 4, 6, 7):
        w = inputs[f"w{l}"][e]
        o = s * 512
        for k in (0, 1):
            reg[f"w{l}"][:, o + k * 256: o + (k + 1) * 256] = w[128 * k: 128 * (k + 1)]
        set_b2(l, inputs[f"b{l}"][e])
    o = s * 512
    for k in (0, 1):
        reg["w5"][:, o + k * 256: o + (k + 1) * 256] = w5[128 * k: 128 * (k + 1)]
    set_b2(5, inputs["b5"][e])
    wi = inputs["wi"][e]
    for k in (0, 1):
        reg["wi"][:, o + k * 256: o + (k + 1) * 256] = wi[128 * k: 128 * (k + 1)]
    set_b2(8, inputs["bi"][e])
    wa = inputs["wa"][e][:, 0]
    reg["hd"][:, s * 5] = wa[0:128]
    reg["hd"][:, s * 5 + 1] = wa[128:256]
    bt[0, 18 * nslot + s] = inputs["ba"][e][0]
    wc0 = inputs["wc0"][e]
    reg["wc0"][:, s * 256: s * 256 + 128] = wc0[0:128]
    reg["wc0"][:, s * 256 + 128: s * 256 + 256] = wc0[128:256]
    bt[:, 19 * nslot + s] = inputs["bc0"][e]
    reg["hd"][:, s * 5 + 2: s * 5 + 5] = inputs["wc1"][e]
    bt[0:3, 20 * nslot + s] = inputs["bc1"][e]


# ---------------------------------------------------------------------------
# Device program
# ---------------------------------------------------------------------------

def _build_program(C, nslot):
    nall = nslot * C
    waves = _make_waves(nslot, C)
    nw = len(waves)
    f32 = mybir.dt.float32
    bf16 = mybir.dt.bfloat16
    f8e3 = mybir.dt.float8e3
    Sin = mybir.ActivationFunctionType.Sin
    Sigmoid = mybir.ActivationFunctionType.Sigmoid
    Relu = mybir.ActivationFunctionType.Relu
    ADD = mybir.AluOpType.add
    SUB = mybir.AluOpType.subtract
    MUL = mybir.AluOpType.mult
    MAX = mybir.AluOpType.max
    MIN = mybir.AluOpType.min
    INV2PI = float(np.float32(1.0 / (2 * PI)))
    MAGIC = 12582912.0            # 1.5 * 2**23: forces round-to-int in fp32
    C1 = 6.28125                  # 2*pi high part, exact in fp32
    C2 = float(np.float32(2 * PI - 6.28125))
    CLAMP = 3.1415925             # just under pi (ACT Sin domain is [-pi, pi])
    HALF_PI = float(np.float32(PI / 2))
    DESC = float(1.0 / FP8_SCALE)

    def rdt(r):
        return f8e3 if r in FP8R else bf16

    nc = bacc.Bacc("TRN2", target_bir_lowering=False, debug=False)
    wt_d = {}
    for r in REGION_ORDER:
        rows, cols = REGIONS[r]
        wt_d[r] = nc.dram_tensor(f"wt_{r}", (rows, nslot * cols), rdt(r),
                                 kind="ExternalInput")
    bt_d = nc.dram_tensor("bt", (128, NB * nslot), f32, kind="ExternalInput")
    # sm3 rows: [fx 0:18 | fd 18:30 | ptsT 30:30+nall | dirT ...+nall]
    sm3_d = nc.dram_tensor("sm3", (3, 30 + 2 * nall), f32, kind="ExternalInput")
    xyz_d = nc.dram_tensor("xyzb", (6, nall), bf16, kind="ExternalInput")
    al_d = nc.dram_tensor("alpha_out", (1, nall), f32, kind="ExternalOutput")
    co_d = nc.dram_tensor("color_out", (3, nall), f32, kind="ExternalOutput")

    with tile.TileContext(nc) as tc:
        with (
            tc.tile_pool(name="cp", bufs=1) as cp,
            tc.tile_pool(name="xp", bufs=2 * nw + 2) as xp,
            tc.tile_pool(name="psA", bufs=6, space=bass.MemorySpace.PSUM) as psA,
            tc.tile_pool(name="psB", bufs=2, space=bass.MemorySpace.PSUM) as psB,
        ):
            embP = cp.tile([53, nall], bf16)   # sin 0:18, cos 32:50, xyz 50:53
            embD = cp.tile([47, nall], bf16)   # sin 0:12, cos 32:44, xyz 44:47
            nc.vector.memset(embP[:], 0.0)
            nc.vector.memset(embD[:], 0.0)
            # small inputs: scalar ring first, then weight pieces on both rings
            sm3 = cp.tile([3, 30 + 2 * nall], f32)
            nc.scalar.dma_start(sm3[:], sm3_d.ap()[:])
            nc.scalar.dma_start(embP[50:53, :], xyz_d.ap()[0:3, :])
            nc.scalar.dma_start(embD[44:47, :], xyz_d.ap()[3:6, :])
            bt_sb = cp.tile([128, NB * nslot], f32)
            nc.scalar.dma_start(bt_sb[:], bt_d.ap()[:])
            fx_sb = sm3[:, 0:18]
            fd_sb = sm3[:, 18:30]
            pts_sb = sm3[:, 30:30 + nall]
            dir_sb = sm3[:, 30 + nall:30 + 2 * nall]

            # ---- weight pieces, consumption order, both rings balanced ----
            wts = {}
            for r in REGION_ORDER:
                rows, cols = REGIONS[r]
                wts[r] = cp.tile([rows, nslot * cols], rdt(r),
                                 name=f"wt_{r}", tag=f"wt_{r}")
            # all weight DMAs on the sync HWDGE ring, in consumption order:
            # the scalar engine must stay free for ACT work (Sin/Relu), and a
            # single InstDMACopy already spreads across all 16 SDMA engines.
            for r in REGION_ORDER:
                nc.sync.dma_start(wts[r][:], wt_d[r].ap()[:])

            def slab(r, s, lo, hi, wv, rows=128):
                cols = REGIONS[r][1]
                o = s * cols
                return wts[r][0:rows, o + lo: o + hi]

            alpha_sb = cp.tile([1, nall], f32)
            color_sb = cp.tile([3, nall], f32)

            # frequency expansion + range-reduced sin/cos, in <=512-col chunks
            def reduce_sin(tsrc, rows, ncol):
                t1 = xp.tile([rows, ncol], f32, tag="vred")
                nc.vector.tensor_scalar(t1[:], tsrc, INV2PI, MAGIC, MUL, ADD)
                r = xp.tile([rows, ncol], f32, tag="vred")
                nc.vector.tensor_scalar(r[:], t1[:], MAGIC, None, SUB)
                rd = xp.tile([rows, ncol], f32, tag="vred")
                nc.vector.scalar_tensor_tensor(rd[:], r[:], -C1, tsrc, MUL, ADD)
                rd2 = xp.tile([rows, ncol], f32, tag="vred")
                nc.vector.scalar_tensor_tensor(rd2[:], r[:], -C2, rd[:], MUL, ADD)
                v = xp.tile([rows, ncol], f32, tag="vred")
                nc.vector.tensor_scalar(v[:], rd2[:], CLAMP, -CLAMP, MIN, MAX)
                return v

            for lo in range(0, nall, 512):
                hi = min(nall, lo + 512)
                w_ = hi - lo
                for (rows, fmat, src, dst) in (
                    (18, fx_sb, pts_sb, embP),
                    (12, fd_sb, dir_sb, embD),
                ):
                    ep = psA.tile([rows, w_], f32, tag="mlp")
                    nc.tensor.matmul(ep[:], fmat[:, 0:rows], src[:, lo:hi],
                                     start=True, stop=True)
                    vs = reduce_sin(ep[:], rows, w_)
                    nc.scalar.activation(dst[0:rows, lo:hi], vs[:], Sin)
                    pre = xp.tile([rows, w_], f32, tag="vred")
                    nc.vector.tensor_scalar(pre[:], ep[:], HALF_PI, None, ADD)
                    vc = reduce_sin(pre[:], rows, w_)
                    cs = xp.tile([rows, w_], f32, tag="vred")
                    nc.scalar.activation(cs[:], vc[:], Sin)
                    nc.vector.tensor_copy(dst[32:32 + rows, lo:hi], cs[:])

            # ---- wave-lockstep MLP ----
            def bias2_bcast(lidx, s0, s1):
                g = s1 - s0
                ap = bt_sb[:, lidx * 2 * nslot + s0 * 2: lidx * 2 * nslot + s1 * 2]
                return ap.broadcast_to([128, 2 * g, C])

            def bias1_bcast(which, s0, s1, p=128):
                g = s1 - s0
                ap = bt_sb[0:p, which * nslot + s0: which * nslot + s1]
                return ap.broadcast_to([p, g, C])

            xs = [None] * nw
            its = [None] * nw
            cts = [None] * nw

            def mm_mid(r, ps, xin, s0, s1, wv):
                for i in range(s1 - s0):
                    s = s0 + i
                    for j in (0, 1):
                        pj = ps[:, j, i * C:(i + 1) * C]
                        nc.tensor.matmul(pj, slab(r, s, j * 128, j * 128 + 128, wv),
                                         xin[:, 0, i * C:(i + 1) * C],
                                         start=True, stop=False)
                        nc.tensor.matmul(pj, slab(r, s, 256 + j * 128, 256 + j * 128 + 128, wv),
                                         xin[:, 1, i * C:(i + 1) * C],
                                         start=False, stop=True)

            def move2(ps, lidx, s0, s1, wv, relu=True, fp8=False):
                g = s1 - s0
                xn = xp.tile([128, 2, g * C], bf16, tag="x")
                psv = ps[:].rearrange("p j (g c) -> p (j g) c", g=g)
                xnv = xn[:].rearrange("p j (g c) -> p (j g) c", g=g)
                if fp8:
                    nc.vector.scalar_tensor_tensor(
                        xnv, psv, DESC, bias2_bcast(lidx, s0, s1), MUL, ADD)
                else:
                    nc.vector.tensor_tensor(xnv, psv, bias2_bcast(lidx, s0, s1), ADD)
                if relu:
                    nc.scalar.activation(xn[:], xn[:], Relu)
                return xn

            def emit_stage(wv, stage):
                s0, s1 = waves[wv]
                g = s1 - s0
                if stage == 0:  # L0
                    ps = psA.tile([128, 2, g * C], f32, tag="mlp")
                    for i in range(g):
                        s = s0 + i
                        sl = slice(s * C, (s + 1) * C)
                        for j in (0, 1):
                            nc.tensor.matmul(ps[:, j, i * C:(i + 1) * C],
                                             slab("l0", s, j * 128, j * 128 + 128, wv, rows=53),
                                             embP[0:53, sl],
                                             start=True, stop=True)
                    xs[wv] = move2(ps, 0, s0, s1, wv, fp8="l0" in FP8R)
                elif stage in (1, 2, 3, 4, 6, 7):
                    r = f"w{stage}"
                    ps = psA.tile([128, 2, g * C], f32, tag="mlp")
                    mm_mid(r, ps, xs[wv], s0, s1, wv)
                    xs[wv] = move2(ps, stage, s0, s1, wv, fp8=r in FP8R)
                elif stage == 5:
                    ps = psA.tile([128, 2, g * C], f32, tag="mlp")
                    xin = xs[wv]
                    for i in range(g):
                        s = s0 + i
                        sl = slice(s * C, (s + 1) * C)
                        for j in (0, 1):
                            pj = ps[:, j, i * C:(i + 1) * C]
                            nc.tensor.matmul(pj, slab("w5", s, j * 128, j * 128 + 128, wv),
                                             xin[:, 0, i * C:(i + 1) * C],
                                             start=True, stop=False)
                            nc.tensor.matmul(pj, slab("w5", s, 256 + j * 128, 256 + j * 128 + 128, wv),
                                             xin[:, 1, i * C:(i + 1) * C],
                                             start=False, stop=False)
                            nc.tensor.matmul(pj, slab("skip", s, j * 128, j * 128 + 128, wv, rows=53),
                                             embP[0:53, sl],
                                             start=False, stop=True)
                    xs[wv] = move2(ps, 5, s0, s1, wv, fp8="w5" in FP8R)
                elif stage == 8:  # wi -> inter (bias, no relu)
                    ps = psA.tile([128, 2, g * C], f32, tag="mlp")
                    mm_mid("wi", ps, xs[wv], s0, s1, wv)
                    its[wv] = move2(ps, 8, s0, s1, wv, relu=False, fp8="wi" in FP8R)
                elif stage == 9:  # wa -> alpha
                    pa = psB.tile([3, g * C], f32, tag="head")
                    xin = xs[wv]
                    for i in range(g):
                        s = s0 + i
                        nc.tensor.matmul(pa[0:1, i * C:(i + 1) * C],
                                         slab("hd", s, 0, 1, wv),
                                         xin[:, 0, i * C:(i + 1) * C],
                                         start=True, stop=False)
                        nc.tensor.matmul(pa[0:1, i * C:(i + 1) * C],
                                         slab("hd", s, 1, 2, wv),
                                         xin[:, 1, i * C:(i + 1) * C],
                                         start=False, stop=True)
                    av = alpha_sb[0:1, s0 * C: s1 * C].rearrange(
                        "p (g c) -> p g c", g=g)
                    pav = pa[0:1, :].rearrange("p (g c) -> p g c", g=g)
                    nc.vector.tensor_tensor(av, pav, bias1_bcast(18, s0, s1, p=1), ADD)
                elif stage == 10:  # wc0 + rays -> c (relu)
                    pc = psA.tile([128, g * C], f32, tag="mlp")
                    it = its[wv]
                    for i in range(g):
                        s = s0 + i
                        sl = slice(s * C, (s + 1) * C)
                        pj = pc[:, i * C:(i + 1) * C]
                        nc.tensor.matmul(pj, slab("wc0", s, 0, 128, wv),
                                         it[:, 0, i * C:(i + 1) * C],
                                         start=True, stop=False)
                        nc.tensor.matmul(pj, slab("wc0", s, 128, 256, wv),
                                         it[:, 1, i * C:(i + 1) * C],
                                         start=False, stop=False)
                        nc.tensor.matmul(pj, slab("rays", s, 0, 128, wv, rows=47),
                                         embD[0:47, sl],
                                         start=False, stop=True)
                    ct = xp.tile([128, g * C], bf16, tag="ct")
                    pcv = pc[:].rearrange("p (g c) -> p g c", g=g)
                    ctv = ct[:].rearrange("p (g c) -> p g c", g=g)
                    if "wc0" in FP8R:
                        nc.vector.scalar_tensor_tensor(
                            ctv, pcv, DESC, bias1_bcast(19, s0, s1), MUL, ADD)
                    else:
                        nc.vector.tensor_tensor(ctv, pcv, bias1_bcast(19, s0, s1), ADD)
                    nc.scalar.activation(ct[:], ct[:], Relu)
                    cts[wv] = ct
                elif stage == 11:  # wc1 -> sigmoid color
                    pcol = psB.tile([3, g * C], f32, tag="head")
                    ct = cts[wv]
                    for i in range(g):
                        s = s0 + i
                        nc.tensor.matmul(pcol[:, i * C:(i + 1) * C],
                                         slab("hd", s, 2, 5, wv),
                                         ct[:, i * C:(i + 1) * C],
                                         start=True, stop=True)
                    ctmp = xp.tile([3, g * C], f32, tag="ctmp")
                    pv = pcol[:].rearrange("p (g c) -> p g c", g=g)
                    cv = ctmp[:].rearrange("p (g c) -> p g c", g=g)
                    nc.vector.tensor_tensor(cv, pv, bias1_bcast(20, s0, s1, p=3), ADD)
                    nc.scalar.activation(color_sb[0:3, s0 * C: s1 * C], ctmp[:],
                                         Sigmoid)

            for stage in range(12):
                for wv in range(nw):
                    emit_stage(wv, stage)

            nc.sync.dma_start(al_d.ap()[:], alpha_sb[:])
            nc.scalar.dma_start(co_d.ap()[:], color_sb[:])

    nc.compile()
    return nc


_prog_cache = {}
_last_results = None


def _get_program(C, nslot):
    key = (C, nslot)
    if key not in _prog_cache:
        _prog_cache[key] = _build_program(C, nslot)
    return _prog_cache[key]


# ---------------------------------------------------------------------------
# Host wrapper
# ---------------------------------------------------------------------------

def kernel(**inputs):
    global _last_results
    inputs = {k: np.asarray(v) for k, v in inputs.items()}
    idx = inputs["index"].astype(np.int64)
    B = idx.shape[0]
    points = inputs["points"].astype(np.float32)
    dirs = inputs["directions"].astype(np.float32)

    # --- routing: split each expert's tokens into <=CAP_MAX chunks, round-
    # robin (sorted by size) over 8 cores ---
    tok = [np.nonzero(idx == e)[0] for e in range(E)]
    virt = []
    for e in range(E):
        t = tok[e]
        if len(t) == 0:
            continue
        for lo in range(0, len(t), CAP_MAX):
            virt.append((e, t[lo: lo + CAP_MAX]))
    if not virt:
        virt = [(0, np.zeros((0,), np.int64))]
    virt.sort(key=lambda v: -len(v[1]))
    nslot = max(1, int(np.ceil(len(virt) / N_CORES)))
    C = max(4, int(np.ceil(max(len(v[1]) for v in virt) / 4) * 4))
    nall = nslot * C

    core_slots = [[] for _ in range(N_CORES)]
    for i, v in enumerate(virt):
        core_slots[i % N_CORES].append(v)
    waves = _make_waves(nslot, C)

    nc = _get_program(C, nslot)

    fx = np.zeros((3, 18), np.float32)
    for c in range(3):
        for k in range(NX):
            fx[c, c * NX + k] = float(2 ** k)
    fd = np.zeros((3, 12), np.float32)
    for c in range(3):
        for k in range(ND):
            fd[c, c * ND + k] = float(2 ** k)

    in_maps = []
    for c in range(N_CORES):
        reg = {r: np.zeros((REGIONS[r][0], nslot * REGIONS[r][1]), np.float32)
               for r in REGION_ORDER}
        bt = np.zeros((128, NB * nslot), np.float32)
        ptsT = np.zeros((3, nall), np.float32)
        dirT = np.zeros((3, nall), np.float32)
        for s, (e, t) in enumerate(core_slots[c]):
            _pack_expert(reg, bt, s, nslot, inputs, e, waves)
            n = len(t)
            if n:
                ptsT[:, s * C: s * C + n] = points[t].T
                dirT[:, s * C: s * C + n] = dirs[t].T
        sm3 = np.concatenate([fx, fd, ptsT, dirT], axis=1)
        xyzb = np.concatenate([ptsT, dirT], axis=0).astype(ml_dtypes.bfloat16)
        im = {"bt": bt, "sm3": sm3, "xyzb": xyzb}
        for r in REGION_ORDER:
            if r in FP8R:
                im[f"wt_{r}"] = (reg[r] * FP8_SCALE).astype(ml_dtypes.float8_e3m4)
            elif r in SCALED_BF16:
                im[f"wt_{r}"] = (reg[r] * FP8_SCALE).astype(ml_dtypes.bfloat16)
            else:
                im[f"wt_{r}"] = reg[r].astype(ml_dtypes.bfloat16)
        in_maps.append(im)

    res = run_bass_kernel_spmd(nc, in_maps, core_ids=list(range(N_CORES)))
    _last_results = res

    out = np.zeros((B, 4), np.float32)
    for c in range(N_CORES):
        al = res.results[c]["alpha_out"]
        co = res.results[c]["color_out"]
        for s, (e, t) in enumerate(core_slots[c]):
            n = len(t)
            if n:
                out[t, 0] = al[0, s * C: s * C + n]
                out[t, 1:4] = co[:, s * C: s * C + n].T
    return out
